# revision 1
# baseline (speedup 1.0000x reference)
"""Multi-head causal attention (B=4, S=2048, D=1024, H=16) on 8 Trainium2 cores.

Sharding: batch x head-group. Core c handles batch c//2 and head-group c%2
(8 heads = 512 features). wq/wk/wv are split column-wise (in x @ w.T terms),
wo row-wise; each pair of cores produces a partial [S, D] output for its batch
which is reduced on the host (the "all-reduce after the output projection").

Device kernel per core (identical SPMD program, inputs pre-sliced/transposed
and rounded to bf16 on host; all matmuls bf16 with fp32 PSUM accumulation):
  - V computed first as [s, f] with a ones-column per head (the PV matmul then
    also produces softmax row-sums), then Q^T/K^T as [f, s] per feature tile so
    attention for early heads overlaps the remaining projections.
  - Attention per head h, sq-half, key-tile j: scores^T [sk, sq] from a K=64
    matmul (columns left of the diagonal never computed), exp on ScalarE with
    fused 1/8 scale straight from PSUM (no max subtraction: scores ~ N(0,1),
    overflow impossible in fp32), causal triangle handled by a post-exp 0/1
    multiply on the 128-wide diagonal block, PV accumulates out^T [65, 1024]
    in PSUM with the diagonal chunk narrowed to skip masked columns.
  - Normalization batched at the end: one ScalarE Reciprocal over all row-sums,
    partition-broadcast via a DRAM bounce, in-place bf16 multiply per head.
  - Output projection row-parallel over 128-row tiles, PSUM accumulation over
    the 4 local feature tiles.
"""

import ml_dtypes
import numpy as np

import concourse.bass as bass
import concourse.mybir as mybir
import concourse.tile as tile
from concourse import bacc
from concourse.bass_utils import run_bass_kernel_spmd

B, S, D, H, HD = 4, 2048, 1024, 16, 64
NCORES = 8
FL = 512          # local features per core (8 heads)
NH = 8            # local heads per core
ND = D // 128     # 8 contraction tiles
NFT = FL // 128   # 4 local feature tiles
NST = S // 128    # 16 sequence tiles

F32 = mybir.dt.float32
BF16 = mybir.dt.bfloat16
EXP = mybir.ActivationFunctionType.Exp
RECIP = mybir.ActivationFunctionType.Reciprocal

BF = ml_dtypes.bfloat16

_CACHE = {}


def _build():
    nc = bacc.Bacc("TRN2", target_bir_lowering=False, debug=False)

    xT = nc.declare_dram_parameter("xT", [D, S], BF16, isOutput=False)
    wqT = nc.declare_dram_parameter("wqT", [D, FL], BF16, isOutput=False)
    wkT = nc.declare_dram_parameter("wkT", [D, FL], BF16, isOutput=False)
    wvT = nc.declare_dram_parameter("wvT", [D, FL], BF16, isOutput=False)
    woT = nc.declare_dram_parameter("woT", [FL, D], BF16, isOutput=False)
    tri01 = nc.declare_dram_parameter("tri01", [128, 128], BF16, isOutput=False)
    ones8 = nc.declare_dram_parameter("ones8", [128, 8], BF16, isOutput=False)
    out = nc.declare_dram_parameter("out", [S, D], F32, isOutput=True)

    xT_t = xT[:].rearrange("(t p) s -> t p s", p=128)
    wqT_t = wqT[:].rearrange("(t p) f -> t p f", p=128)
    wkT_t = wkT[:].rearrange("(t p) f -> t p f", p=128)
    wvT_t = wvT[:].rearrange("(t p) f -> t p f", p=128)
    woT_t = woT[:].rearrange("(t p) o -> t p o", p=128)
    out_t = out[:].rearrange("(t p) o -> t p o", p=128)

    with tile.TileContext(nc) as tc:
        with (
            tc.tile_pool(name="qt", bufs=NFT) as qt_p,
            tc.tile_pool(name="kt", bufs=NFT) as kt_p,
            tc.tile_pool(name="vt", bufs=NST) as vt_p,
            tc.tile_pool(name="msk", bufs=1) as msk_p,
            tc.tile_pool(name="ru", bufs=1) as ru_p,
            tc.tile_pool(name="ps2", bufs=2, space="PSUM") as ps2,
            tc.tile_pool(name="scp", bufs=2, space="PSUM") as scp,
            tc.tile_pool(name="outq", bufs=2, space="PSUM") as outq,
        ):
            tri_sb = msk_p.tile([128, 128], BF16, tag="msk")

            qt = [qt_p.tile([128, S], BF16, tag="qt", name="qt") for _ in range(NFT)]
            kt = [kt_p.tile([128, S], BF16, tag="kt", name="kt") for _ in range(NFT)]
            vt = [vt_p.tile([128, NH * 65], BF16, tag="vt", name="vt") for _ in range(NST)]
            for st in range(NST):
                v3 = vt[st][:].rearrange("p (h c) -> p h c", c=65)
                nc.vector.memset(v3[:, :, 64], 1.0)

            rs = ru_p.tile([65, 1024], F32, tag="rs", name="rs")

            # ---- single interleaved flow: V, then KQ(ft) woven into attention ----
            with (
                tc.tile_pool(name="xt", bufs=1) as xt_p,
                tc.tile_pool(name="wi", bufs=1) as w_p,
                tc.tile_pool(name="at", bufs=NFT) as at_p,
                tc.tile_pool(name="pt", bufs=4) as pt_p,
                tc.tile_pool(name="rep", bufs=2) as rep_p,
                tc.tile_pool(name="wo", bufs=1) as wo_p,
                tc.tile_pool(name="stg", bufs=3) as stg_p,
                tc.tile_pool(name="dbn", bufs=2, space="DRAM") as dbn_p,
            ):
                wv_all = w_p.tile([128, ND * FL], BF16, tag="wv", name="wv")
                wk_all = w_p.tile([128, ND * FL], BF16, tag="wk", name="wk")
                wq_all = w_p.tile([128, ND * FL], BF16, tag="wq", name="wq")
                x_c = [
                    xt_p.tile([128, ND * 512], BF16, tag="xt", name="xt", bufs=4)
                    for _ in range(4)
                ]
                nc.sync.dma_start(
                    wv_all[:].rearrange("p (d f) -> p d f", f=FL),
                    wvT[:].rearrange("(d p) f -> p d f", p=128),
                )
                nc.sync.dma_start(
                    x_c[0][:].rearrange("p (d s) -> p d s", s=512),
                    xT[:, 0:512].rearrange("(d p) s -> p d s", p=128),
                )
                nc.sync.dma_start(
                    wk_all[:].rearrange("p (d f) -> p d f", f=FL),
                    wkT[:].rearrange("(d p) f -> p d f", p=128),
                )
                nc.sync.dma_start(
                    wq_all[:].rearrange("p (d f) -> p d f", f=FL),
                    wqT[:].rearrange("(d p) f -> p d f", p=128),
                )
                for cc in range(1, 4):
                    nc.sync.dma_start(
                        x_c[cc][:].rearrange("p (d s) -> p d s", s=512),
                        xT[:, cc * 512:(cc + 1) * 512].rearrange(
                            "(d p) s -> p d s", p=128
                        ),
                    )
                nc.sync.dma_start(tri_sb[:], tri01[:])
                wo_all = wo_p.tile([128, NFT * D], BF16, tag="wo", name="wo")
                nc.sync.dma_start(
                    wo_all[:].rearrange("p (t o) -> p t o", o=D),
                    woT[:].rearrange("(t p) o -> p t o", p=128),
                )

                at = [at_p.tile([128, S], BF16, tag="at", name="at") for _ in range(NFT)]
                # rowsums reshaped [32, 128] per ft: partition 32*ft+16*sub+8*half+p
                # holds r_h[1024*half + 128*p + c] at column c
                ru32 = ru_p.tile([128, 128], F32, tag="ru", name="ru")
                ri32 = ru_p.tile([128, 128], F32, tag="ri", name="ri")
                rb32 = ru_p.tile([128, 128], BF16, tag="rb", name="rb")

                # V: [s, f] = x @ wv_local^T, written with per-head stride 65
                def v_group(st):
                    ps = ps2.tile([128, 512], F32, tag="ps2", name="vps")
                    for d in range(ND):
                        nc.tensor.matmul(
                            ps[:],
                            x_c[st // 4][
                                :, d * 512 + (st % 4) * 128:d * 512 + (st % 4 + 1) * 128
                            ],
                            wv_all[:, d * FL:(d + 1) * FL],
                            start=(d == 0),
                            stop=(d == ND - 1),
                        )
                    dstv = vt[st][:].rearrange("p (h c) -> p h c", c=65)[:, :, 0:64]
                    srcv = ps[:].rearrange("p (h c) -> p h c", c=64)
                    nc.scalar.copy(dstv, srcv)

                def kq_group(ft, which, c, on_scalar=False):
                    wsb, dst = ((wk_all, kt), (wq_all, qt))[which]
                    ps = ps2.tile([128, 512], F32, tag="ps2", name="kq")
                    for d in range(ND):
                        nc.tensor.matmul(
                            ps[:],
                            wsb[:, d * FL + ft * 128:d * FL + (ft + 1) * 128],
                            x_c[c][:, d * 512:(d + 1) * 512],
                            start=(d == 0),
                            stop=(d == ND - 1),
                        )
                    if on_scalar:
                        nc.scalar.copy(dst[ft][:, c * 512:(c + 1) * 512], ps[:])
                    else:
                        nc.vector.tensor_copy(dst[ft][:, c * 512:(c + 1) * 512], ps[:])

                def attn_quarter(ft, qc, nxt):
                    """Both heads of ft, sq-quarter qc: paired scores matmuls
                    (PE row-groups 0-63 / 64-127 run concurrently), one wide
                    exp over both heads, per-head PV into [65, 512] psum."""
                    q0 = 512 * qc
                    outX = [
                        outq.tile([65, 512], F32, tag="outq", name="outq")
                        for _ in range(2)
                    ]
                    for j in range(4 * qc + 4):
                        diag = j >= 4 * qc
                        off = 128 * j - q0 if diag else 0
                        sct = scp.tile([128, 1024], F32, tag="scp", name="sct")
                        for ro, cb in ((0, 0), (64, 512)):
                            nc.tensor.matmul(
                                sct[:, cb + off:cb + 512],
                                kt[ft][ro:ro + 64, j * 128:(j + 1) * 128],
                                qt[ft][ro:ro + 64, q0 + off:q0 + 512],
                                start=True,
                                stop=True,
                            )
                        ptile = pt_p.tile([128, 1024], BF16, tag="pt", name="pt")
                        for cb in (0, 512):
                            nc.scalar.activation(
                                ptile[:, cb + off:cb + 512],
                                sct[:, cb + off:cb + 512],
                                EXP,
                                scale=0.125,
                            )
                        if diag:
                            for cb in (0, 512):
                                nc.vector.tensor_mul(
                                    ptile[:, cb + off:cb + off + 128],
                                    ptile[:, cb + off:cb + off + 128],
                                    tri_sb[:],
                                )
                        for sub, cb in ((0, 0), (1, 512)):
                            h = 2 * ft + sub
                            nc.tensor.matmul(
                                outX[sub][:, off:512],
                                vt[j][:, h * 65:h * 65 + 65],
                                ptile[:, cb + off:cb + 512],
                                start=(j == 0),
                                stop=(j == 4 * qc + 3),
                            )
                    # quarter epilogue: raw copies + rowsum extraction
                    for sub in range(2):
                        ro = 64 * sub
                        nc.vector.tensor_copy(
                            at[ft][ro:ro + 64, q0:q0 + 512], outX[sub][0:64, :]
                        )
                        nc.vector.tensor_copy(
                            rs[64:65, 512 * sub:512 * sub + 512], outX[sub][64:65, :]
                        )
                        p0 = 32 * ft + 16 * sub + 4 * qc
                        drr = dbn_p.tile([1, 512], F32, tag="drr", name="drr")
                        nc.sync.dma_start(
                            drr[:], rs[64:65, 512 * sub:512 * sub + 512]
                        )
                        nc.sync.dma_start(
                            ru32[p0:p0 + 4, :],
                            drr[:].rearrange("o (p c) -> (o p) c", c=128),
                        )
                    for _ in range(2):
                        if nxt:
                            kq_group(*nxt.pop(0))

                for ft in range(NFT):
                    nxt = (
                        [(ft + 1, w, c) for w in (0, 1) for c in range(4)]
                        if ft < NFT - 1
                        else []
                    )
                    for qc in range(4):
                        if ft == 0:
                            # staircase ramp: V tiles + K0/Q0 chunk for this
                            # quarter only, so attention starts ~25us in
                            for stl in range(4):
                                v_group(4 * qc + stl)
                            kq_group(0, 0, qc, on_scalar=True)
                            kq_group(0, 1, qc, on_scalar=True)
                        attn_quarter(ft, qc, nxt)
                    while nxt:
                        kq_group(*nxt.pop(0))

                    # normalize this feature tile (overlaps next ft's attention)
                    nc.vector.reciprocal(
                        ri32[32 * ft:32 * ft + 32, :], ru32[32 * ft:32 * ft + 32, :]
                    )
                    with nc.allow_low_precision(reason="softmax 1/rowsum to bf16"):
                        nc.vector.tensor_copy(
                            rb32[32 * ft:32 * ft + 32, :],
                            ri32[32 * ft:32 * ft + 32, :],
                        )
                    rep = rep_p.tile([128, S], BF16, tag="rep", name="rep")
                    for sub in range(2):
                        bounce = dbn_p.tile([1, S], BF16, tag="dbn", name="dbn")
                        nc.sync.dma_start(
                            bounce[:].rearrange("o (p c) -> (o p) c", c=128),
                            rb32[32 * ft + 16 * sub:32 * ft + 16 * sub + 16, :],
                        )
                        nc.sync.dma_start(
                            rep[64 * sub:64 * sub + 64, :],
                            bounce[:].to_broadcast((64, S)),
                        )
                    nc.vector.tensor_mul(at[ft][:], at[ft][:], rep[:])

                # output projection: out[s, :] = sum_f at[f, s] * woT[f, :]
                for st in range(NST):
                    for oc in range(2):
                        if (2 * st + oc) % 2 == 0:
                            po = ps2.tile([128, 512], F32, tag="ps2", name="po")
                        else:
                            po = scp.tile([128, 512], F32, tag="scp", name="po")
                        for ft in range(NFT):
                            nc.tensor.matmul(
                                po[:],
                                at[ft][:, st * 128:(st + 1) * 128],
                                wo_all[:, ft * D + oc * 512:ft * D + (oc + 1) * 512],
                                start=(ft == 0),
                                stop=(ft == NFT - 1),
                            )
                        so = stg_p.tile([128, 512], F32, tag="stg", name="stg")
                        nc.scalar.copy(so[:], po[:])
                        nc.sync.dma_start(
                            out_t[st][:, oc * 512:(oc + 1) * 512], so[:]
                        )

    nc.compile()
    return nc


def kernel(x, wq, wk, wv, wo, _trace=False):
    x = np.asarray(x, dtype=np.float32)
    wq = np.asarray(wq, dtype=np.float32)
    wk = np.asarray(wk, dtype=np.float32)
    wv = np.asarray(wv, dtype=np.float32)
    wo = np.asarray(wo, dtype=np.float32)

    if "nc" not in _CACHE:
        _CACHE["nc"] = _build()
    nc = _CACHE["nc"]

    r = np.arange(128)
    tri = (r[None, :] >= r[:, None]).astype(BF)  # keep where sq >= sk
    ones = np.ones((128, 8), dtype=BF)
    in_maps = []
    for c in range(NCORES):
        b, g = c // 2, c % 2
        fsl = slice(g * FL, (g + 1) * FL)
        in_maps.append(
            {
                "xT": np.ascontiguousarray(x[b].T).astype(BF),
                "wqT": np.ascontiguousarray(wq[fsl, :].T).astype(BF),
                "wkT": np.ascontiguousarray(wk[fsl, :].T).astype(BF),
                "wvT": np.ascontiguousarray(wv[fsl, :].T).astype(BF),
                "woT": np.ascontiguousarray(wo[:, fsl].T).astype(BF),
                "tri01": tri,
                "ones8": ones,
            }
        )

    res = run_bass_kernel_spmd(nc, in_maps, list(range(NCORES)), trace=_trace)
    outs = res.results
    full = np.empty((B, S, D), dtype=np.float32)
    for b in range(B):
        full[b] = outs[2 * b]["out"] + outs[2 * b + 1]["out"]
    if _trace:
        return full, res
    return full

